# revision 5
# baseline (speedup 1.0000x reference)
"""Trainium2 Bass kernel for nn_AttentiveTransformer
(fc -> GhostBN -> *prior -> sparsemax), 8-core data-parallel over batch.

Matmul runs in fp8(e4m3) DoubleRow perf mode with 3-term error
compensation:  h ~= x8@W8 + dx8@(W8/16) + x8@(dW8/16), where
x8 = e4m3(x*SX), dx8 = e4m3((x*SX - x8)*16) and likewise for W.
The common scale SX*SW cancels inside GhostBN.  Intermediates (H, prior,
z) are fp16; stats / psum / output are f32.  End-to-end rel-l2 vs the
f32 reference simulates to ~4e-3 (gate 2e-2).

kernel(**inputs) takes FULL f32 inputs, returns FULL f32 output.
"""
import numpy as np
import ml_dtypes
from contextlib import ExitStack

import concourse.bacc as bacc
import concourse.tile as tile
import concourse.mybir as mybir
from concourse.bass_utils import run_bass_kernel_spmd
from concourse import masks

f32 = mybir.dt.float32
fp16 = mybir.dt.float16
fp8 = mybir.dt.float8e4
AF = mybir.ActivationFunctionType
ALU = mybir.AluOpType
AX = mybir.AxisListType
PM = mybir.MatmulPerfMode

N_CORES = 8
B_FULL = 16384
D = 2048                  # D_in == D_out
BL = B_FULL // N_CORES    # 2048 rows per core
P = 128
KP = 8                    # k-pairs (each covers 256 of D_in)
MT = 16                   # m-tiles of 128 over D_out
SEG = 256                 # batch rows per segment
NSEG = BL // SEG          # 8
NIT = 5                   # Newton iterations on compacted candidates
SX, SW = 32.0, 256.0
EPS_RAW = 1e-5 * (SX * SW) ** 2
e4m3 = ml_dtypes.float8_e4m3


def _body(nc, tc, ctx, X8, DX8, Wq, PRI, Gd, Bd, OUT, repeat=1):
    sb_const = ctx.enter_context(tc.tile_pool(name="const", bufs=1))
    wt_pool = ctx.enter_context(tc.tile_pool(name="wt", bufs=1))
    xt_pool = ctx.enter_context(tc.tile_pool(name="xt", bufs=2))
    h_pool = ctx.enter_context(tc.tile_pool(name="h", bufs=2))
    z_pool = ctx.enter_context(tc.tile_pool(name="z", bufs=2))
    zo_pool = ctx.enter_context(tc.tile_pool(name="zo", bufs=4))
    prior_pool = ctx.enter_context(tc.tile_pool(name="prior", bufs=4))
    small_pool = ctx.enter_context(tc.tile_pool(name="small", bufs=2))
    stat_pool = ctx.enter_context(tc.tile_pool(name="stat", bufs=2))
    b6_pool = ctx.enter_context(tc.tile_pool(name="b6", bufs=2))
    mm_ps = ctx.enter_context(tc.tile_pool(name="mm_ps", bufs=6, space="PSUM"))
    zt_ps = ctx.enter_context(tc.tile_pool(name="zt_ps", bufs=2, space="PSUM"))

    # --- constants ---
    ident_f = sb_const.tile([P, P], f32)
    masks.make_identity(nc, ident_f[:])
    ident = sb_const.tile([P, P], fp16)
    nc.vector.tensor_copy(ident[:], ident_f[:])
    eps_t = sb_const.tile([P, 1], f32)
    nc.vector.memset(eps_t[:], EPS_RAW)
    # gamma/beta -> [128, 16] (col m), expanded to [128, 32] (col 2m+v)
    gtmp = sb_const.tile([P, MT], f32)
    btmp = sb_const.tile([P, MT], f32)
    nc.sync.dma_start(gtmp[:], Gd.rearrange("(m p) -> p m", p=P))
    nc.sync.dma_start(btmp[:], Bd.rearrange("(m p) -> p m", p=P))
    gx = sb_const.tile([P, 2 * MT], f32)
    bx = sb_const.tile([P, 2 * MT], f32)
    half = sb_const.tile([P, 2 * MT], f32)
    nc.vector.memset(half[:], 0.5)
    nc.vector.tensor_copy(gx[:, 0:2 * MT:2], gtmp[:])
    nc.vector.tensor_copy(gx[:, 1:2 * MT:2], gtmp[:])
    nc.vector.tensor_copy(bx[:, 0:2 * MT:2], btmp[:])
    nc.vector.tensor_copy(bx[:, 1:2 * MT:2], btmp[:])

    # --- phase 0: load fp8 weight terms (3 terms x 8 kp, 512KB each) ---
    WS = {}
    for t in range(3):
        for kp in range(KP):
            w = wt_pool.tile([P, 2, D], fp8, tag=f"w{t}_{kp}",
                             name=f"w{t}_{kp}")
            eng = nc.scalar if (t * KP + kp) % 2 == 0 else nc.sync
            eng.dma_start(w[:, :, :], Wq[t, kp])
            WS[(t, kp)] = w

    if repeat > 1:
        rep_cm = tc.For_i(0, repeat, 1)
        rep_cm.__enter__()

    def emit_mm_group(s, mg, x8t, dx8t, B6, Hs, SC, SH, msum, dm, vr):
        """4 m-tiles of DR matmuls + bn_stats + stat math + fused apply."""
        pms = []
        for i in range(4):
            m = 4 * mg + i
            pm = mm_ps.tile([P, 2, P], f32, tag="mm", name=f"mm{s}_{mg}_{i}")
            pms.append(pm)
            first = True
            for kp in range(KP):
                for t, mov in ((0, x8t), (1, dx8t), (2, x8t)):
                    nc.tensor.matmul(pm[:, :, :],
                                     WS[(t, kp)][:, :, m * P:(m + 1) * P],
                                     mov[:, kp, :, :],
                                     start=first,
                                     stop=(kp == KP - 1 and t == 2),
                                     perf_mode=PM.DoubleRow)
                    first = False
        # bn_stats per (m, v): 6 outputs = stats of two 64-row halves
        for i in range(4):
            m = 4 * mg + i
            for v in range(2):
                base = 6 * (2 * m + v)
                nc.vector.bn_stats(B6[:, base:base + 6], pms[i][:, v, :])
        # stat math for this group's 8 (m,v) columns c0..c1
        c0, c1 = 8 * mg, 8 * mg + 8
        b0, b1 = 48 * mg, 48 * mg + 48
        me_ap = B6[:, b0 + 1:b1:6]
        mo_ap = B6[:, b0 + 4:b1:6]
        m2e_ap = B6[:, b0 + 2:b1:6]
        m2o_ap = B6[:, b0 + 5:b1:6]
        msum_g = msum[:, c0:c1]
        dm_g = dm[:, c0:c1]
        vr_g = vr[:, c0:c1]
        nc.gpsimd.tensor_tensor(msum_g, me_ap, mo_ap, ALU.add)
        nc.gpsimd.tensor_tensor(dm_g, me_ap, mo_ap, ALU.subtract)
        nc.gpsimd.tensor_tensor(vr_g, m2e_ap, m2o_ap, ALU.add)
        nc.gpsimd.tensor_tensor(dm_g, dm_g, dm_g, ALU.mult)
        nc.vector.scalar_tensor_tensor(vr_g, dm_g, 32.0, vr_g,
                                       ALU.mult, ALU.add)       # var*128
        nc.scalar.activation(vr_g, vr_g, AF.Sqrt, bias=eps_t[:],
                             scale=1.0 / P)                     # std (raw)
        nc.vector.reciprocal(vr_g, vr_g)                        # rstd
        nc.gpsimd.tensor_tensor(SC[:, c0:c1], vr_g, gx[:, c0:c1], ALU.mult)
        nc.gpsimd.tensor_tensor(msum_g, msum_g, half[:, c0:c1], ALU.mult)
        nc.gpsimd.tensor_tensor(msum_g, msum_g, SC[:, c0:c1], ALU.mult)
        nc.gpsimd.tensor_tensor(SH[:, c0:c1], bx[:, c0:c1], msum_g,
                                ALU.subtract)
        # fused apply + evacuate: H[m][:,v] = pm[:,v]*SC + SH  (fp16 out)
        # (GPSIMD can't read PSUM, so all on ACT)
        for i in range(4):
            m = 4 * mg + i
            for v in range(2):
                col = 2 * m + v
                dst = Hs[m][:, v * P:(v + 1) * P]
                nc.scalar.activation(dst, pms[i][:, v, :], AF.Identity,
                                     bias=SH[:, col:col + 1],
                                     scale=SC[:, col:col + 1])

    def emit_transposes(row0, Hs, zs, Cs, q):
        """Transpose quarter q (m = 4q..4q+3) for both u halves, fuse
        prior multiply, top-8 compact."""
        for u in range(2):
            zt = zt_ps.tile([P, 4 * P], fp16, tag="zt")
            for mm_i in range(4):
                m = 4 * q + mm_i
                nc.tensor.transpose(zt[:, mm_i * P:(mm_i + 1) * P],
                                    Hs[m][:, u * P:(u + 1) * P],
                                    ident[:])
            pch = prior_pool.tile([P, 4 * P], fp16, tag="prior")
            nc.sync.dma_start(
                pch[:],
                PRI[row0 + u * P: row0 + (u + 1) * P,
                    q * 4 * P:(q + 1) * 4 * P])
            nc.vector.tensor_tensor(zs[u][:, q * 4 * P:(q + 1) * 4 * P],
                                    zt[:], pch[:], ALU.mult)
            nc.vector.max(Cs[u][:, 8 * q:8 * q + 8],
                          zs[u][:, 512 * q:512 * (q + 1)])

    def make_newton(zs, Cs, row0):
        """Generator yielding Newton-iteration steps (interleaved later)."""
        its = []
        for u in range(2):
            it = small_pool.tile([P, 8], f32, tag="it", name=f"it{row0}_{u}")
            its.append(it)
            nc.vector.tensor_reduce(it[:, 0:1], Cs[u][:, 7:32:8], axis=AX.X,
                                    op=ALU.max, negate=True)     # tneg
            nc.vector.tensor_reduce(it[:, 5:6], Cs[u][:, 7:32:8], axis=AX.X,
                                    op=ALU.max)                  # tpos
        relus = [small_pool.tile([P, 32], fp16, tag="relu",
                                 name=f"rl{row0}_{u}") for u in range(2)]
        signs = [small_pool.tile([P, 32], fp16, tag="sign",
                                 name=f"sg{row0}_{u}") for u in range(2)]

        def step(u):
            it = its[u]
            tneg, racc, kacc = it[:, 0:1], it[:, 1:2], it[:, 2:3]
            krec, delta, tpos = it[:, 3:4], it[:, 4:5], it[:, 5:6]
            nc.scalar.activation(relus[u][:], Cs[u][:], AF.Relu, bias=tneg,
                                 accum_out=racc)
            nc.vector.tensor_scalar(signs[u][:], Cs[u][:], tpos, 0.0,
                                    ALU.is_gt, ALU.add, accum_out=kacc)
            nc.vector.reciprocal(krec, kacc)
            nc.vector.scalar_tensor_tensor(delta, racc, -1.0, krec,
                                           ALU.add, ALU.mult)
            nc.gpsimd.tensor_tensor(tneg, tneg, delta, ALU.subtract)
            nc.gpsimd.tensor_tensor(tpos, tpos, delta, ALU.add)

        def finish(u):
            zo = zo_pool.tile([P, D], f32, tag="zo", name=f"zo{row0}_{u}")
            nc.scalar.activation(zo[:], zs[u][:], AF.Relu,
                                 bias=its[u][:, 0:1])
            nc.sync.dma_start(OUT[row0 + u * P: row0 + (u + 1) * P, :],
                              zo[:])
        return step, finish

    # --- main pipeline over segments ---
    prev = None   # (row0, Hs, zs, Cs) of previous segment
    for s in range(NSEG):
        row0 = s * SEG
        x8t = xt_pool.tile([P, KP, 2, SEG], fp8, tag="x8", name=f"x8_{s}")
        dx8t = xt_pool.tile([P, KP, 2, SEG], fp8, tag="dx8", name=f"dx8_{s}")
        nc.sync.dma_start(x8t[:, :, :, :], X8[s])
        nc.sync.dma_start(dx8t[:, :, :, :], DX8[s])

        B6 = b6_pool.tile([P, 12 * MT], f32, tag="B6", name=f"B6_{s}")
        SC = stat_pool.tile([P, 2 * MT], f32, tag="SC")
        SH = stat_pool.tile([P, 2 * MT], f32, tag="SH")
        msum = stat_pool.tile([P, 2 * MT], f32, tag="msum")
        dm = stat_pool.tile([P, 2 * MT], f32, tag="dm")
        vr = stat_pool.tile([P, 2 * MT], f32, tag="vr")
        Hs = [h_pool.tile([P, SEG], fp16, tag=f"h{m}", name=f"h{s}_{m}")
              for m in range(MT)]
        zs = [z_pool.tile([P, D], fp16, tag=f"z{u}", name=f"z{s}_{u}")
              for u in range(2)]
        Cs = [small_pool.tile([P, 32], fp16, tag=f"C{u}", name=f"C{s}_{u}")
              for u in range(2)]

        if prev is None:
            for mg in range(4):
                emit_mm_group(s, mg, x8t, dx8t, B6, Hs, SC, SH, msum, dm, vr)
        else:
            prow0, pHs, pzs, pCs = prev
            emit_mm_group(s, 0, x8t, dx8t, B6, Hs, SC, SH, msum, dm, vr)
            # previous seg's transposes + prior mult + compaction
            for q in range(4):
                emit_transposes(prow0, pHs, pzs, pCs, q)
            step, finish = make_newton(pzs, pCs, prow0)
            emit_mm_group(s, 1, x8t, dx8t, B6, Hs, SC, SH, msum, dm, vr)
            for itn in range(NIT):
                step(0)
                step(1)
                if itn == 1:
                    emit_mm_group(s, 2, x8t, dx8t, B6, Hs, SC, SH,
                                  msum, dm, vr)
                if itn == 3:
                    emit_mm_group(s, 3, x8t, dx8t, B6, Hs, SC, SH,
                                  msum, dm, vr)
            finish(0)
            finish(1)
        prev = (row0, Hs, zs, Cs)

    # drain last segment
    prow0, pHs, pzs, pCs = prev
    for q in range(4):
        emit_transposes(prow0, pHs, pzs, pCs, q)
    step, finish = make_newton(pzs, pCs, prow0)
    for itn in range(NIT):
        step(0)
        step(1)
    finish(0)
    finish(1)

    if repeat > 1:
        rep_cm.__exit__(None, None, None)


def build(repeat=1):
    nc = bacc.Bacc("TRN2", target_bir_lowering=False, debug=False)
    X8 = nc.dram_tensor("x8", [NSEG, P, KP, 2, SEG], fp8,
                        kind="ExternalInput").ap()
    DX8 = nc.dram_tensor("dx8", [NSEG, P, KP, 2, SEG], fp8,
                         kind="ExternalInput").ap()
    Wq = nc.dram_tensor("Wq", [3, KP, P, 2, D], fp8,
                        kind="ExternalInput").ap()
    PRI = nc.dram_tensor("prior", [BL, D], fp16, kind="ExternalInput").ap()
    Gd = nc.dram_tensor("gamma", [D], f32, kind="ExternalInput").ap()
    Bd = nc.dram_tensor("beta", [D], f32, kind="ExternalInput").ap()
    OUT = nc.dram_tensor("out", [BL, D], f32, kind="ExternalOutput").ap()
    with tile.TileContext(nc) as tc, ExitStack() as ctx:
        _body(nc, tc, ctx, X8, DX8, Wq, PRI, Gd, Bd, OUT, repeat=repeat)
    nc.compile()
    return nc


def _quant_pair(a, scale):
    """a*scale -> (e4m3 main, e4m3 residual*16), as raw fp8 arrays."""
    hi = (a * scale).astype(e4m3)
    lo = ((a * scale - hi.astype(np.float32)) * 16.0).astype(e4m3)
    return hi, lo


def prep_inputs(prior, x, W, gamma, beta):
    """Host-side packing shared by kernel() and test.py."""
    x = np.asarray(x, dtype=np.float32)
    W = np.asarray(W, dtype=np.float32)
    WT = np.ascontiguousarray(W.T)                       # [i, o]
    W8, dW8 = _quant_pair(WT, SW)
    W8_16 = (W8.astype(np.float32) / 16.0).astype(e4m3)
    dW8_16 = (dW8.astype(np.float32) / 16.0).astype(e4m3)
    Wq = np.empty((3, KP, P, 2, D), dtype=e4m3)
    for t, arr in enumerate((W8, W8_16, dW8_16)):
        Wq[t] = arr.reshape(KP, 2, P, D).transpose(0, 2, 1, 3)
    prior16 = np.asarray(prior, dtype=np.float16)
    gamma = np.ascontiguousarray(gamma, dtype=np.float32)
    beta = np.ascontiguousarray(beta, dtype=np.float32)

    in_maps = []
    for c in range(N_CORES):
        sl = slice(c * BL, (c + 1) * BL)
        xT = np.ascontiguousarray(x[sl].T)               # [i, rows]
        x8, dx8 = _quant_pair(xT, SX)
        # i = 256*kp + 128*two + p ; rows = 256*s + b
        x8p = x8.reshape(KP, 2, P, NSEG, SEG).transpose(3, 2, 0, 1, 4)
        dx8p = dx8.reshape(KP, 2, P, NSEG, SEG).transpose(3, 2, 0, 1, 4)
        in_maps.append({"x8": np.ascontiguousarray(x8p),
                        "dx8": np.ascontiguousarray(dx8p),
                        "Wq": Wq,
                        "prior": np.ascontiguousarray(prior16[sl]),
                        "gamma": gamma, "beta": beta})
    return in_maps


_NC = None


def _run(inputs, trace=False, **kw):
    global _NC
    if _NC is None:
        _NC = build()
    in_maps = prep_inputs(inputs["prior"], inputs["x"], inputs["W"],
                          inputs["gamma"], inputs["beta"])
    res = run_bass_kernel_spmd(_NC, in_maps, list(range(N_CORES)),
                               trace=trace, **kw)
    out = np.concatenate([res.results[i]["out"] for i in range(N_CORES)],
                         axis=0)
    return out, res


def kernel(prior, x, W, gamma, beta):
    out, _ = _run({"prior": prior, "x": x, "W": W,
                   "gamma": gamma, "beta": beta})
    return out
